# revision 15
# baseline (speedup 1.0000x reference)
"""Trainium2 Bass kernel for nn_DynamicConvolution (dense_cnn).

Reference computation (per batch of 16 samples):
  pooled = mean(context, HW) -> logits = pooled@attn_w.T + b -> softmax over 4
  dyn_k[b] = sum_n attn[b,n] * kernels[n]          (per-sample 3x3 conv weights)
  out = conv2d(x, dyn_k, SAME)                     (32->32ch, 512x512)
  out = batchnorm(out, batch stats over (B,H,W))

Sharding: data-parallel, 2 samples per NeuronCore across 8 cores. BN batch
statistics are all-reduced on-device (tiny [128,2] collective).

Conv strategy: tap-shift matmuls. x is staged in SBUF as
[128 partitions = 4 W-bands x 32 ch, 18 rows x 130 cols (1-col halo)].
Each of the 9 taps is one K=32 matmul per (W-band i, row-quad j) accumulated
in PSUM via tile_position=(32i,32j) -- 16 sub-array-concurrent matmuls per
tap round, bf16 operands (fp32r forbids col-group tiling; bf16 also
halves the x/ctx HBM traffic -- the host passes x/ctx pre-cast to bf16).

BN is two-pass without storing an intermediate: phase 1 computes conv and
bn_stats only (no output write), stats are all-reduced, phase 2 recomputes
conv and applies the affine BN epilogue on the Scalar engine while copying
PSUM->SBUF, then DMAs out. Total HBM traffic/core ~= ctx 64MB + 2x x-reads
(72MB each, halo) + out 64MB.
"""

import numpy as np

import concourse.bacc as bacc
import concourse.bass as bass
import concourse.mybir as mybir
import concourse.tile as tile
from concourse.bass_utils import run_bass_kernel_spmd

F32 = mybir.dt.float32
BF16 = mybir.dt.bfloat16
AF = mybir.ActivationFunctionType
ALU = mybir.AluOpType

B, C, H, W, NK = 16, 32, 512, 512, 4
BN_EPS = 1e-5
N_CORES = 8
SPC = B // N_CORES          # samples per core
PAD = 516                   # element pad around flat x (OOB-safe halo reads)
RPI = 16                    # output rows per super-iteration
NBAND = 4                   # W-bands (128 cols each)
BW = 130                    # band width incl 1-col halo each side
NR = RPI + 2                # input rows per window


def _build(h=H):
    """Build + compile the 8-core Bacc program. h parametrizes image height
    (h=H for real runs, smaller for simulation tests)."""
    n_iters = h // RPI
    chw = C * h * W
    hw = h * W
    qhw = hw // 4                  # ctx row-quarter size per channel
    n_ctx_tiles = max(1, qhw // 8192)
    ctx_tile_f = qhw // n_ctx_tiles
    n_loc = SPC * n_iters * NBAND * 512   # elements per (quad,ch) stat partition
    n_glob = float(B * h * W)

    nc = bacc.Bacc(
        "TRN2",
        target_bir_lowering=False,
        debug=False,
        enable_asserts=False,
        num_devices=N_CORES,
    )

    x_in = nc.declare_dram_parameter("x", [SPC * chw + 2 * PAD], BF16, isOutput=False)
    ctx_in = nc.declare_dram_parameter("ctx", [SPC * chw], BF16, isOutput=False)
    kern_in = nc.declare_dram_parameter("kern", [C, NK * 9 * C], F32, isOutput=False)
    w4_in = nc.declare_dram_parameter("w4", [128, NK], F32, isOutput=False)
    ab_in = nc.declare_dram_parameter("ab", [NK, 1], F32, isOutput=False)
    gam_in = nc.declare_dram_parameter("gam", [C, 1], F32, isOutput=False)
    bet_in = nc.declare_dram_parameter("bet", [C, 1], F32, isOutput=False)
    out_ext = nc.declare_dram_parameter("out", [SPC * chw], F32, isOutput=True)

    # internal DRAM
    exps_d = nc.dram_tensor("exps_d", [NK], F32)
    recip_d = nc.dram_tensor("recip_d", [1], F32)
    dyn_d = nc.dram_tensor("dyn_d", [C * 9 * C], BF16)
    ss_d = nc.dram_tensor("ss_d", [C, 2], F32)
    ar_in_d = nc.dram_tensor("ar_in", [128, 2], F32)
    ar_out_d = nc.dram_tensor("ar_out", [128, 2], F32, addr_space="Shared")

    def dram_ap(t, offset, ap):
        return bass.AP(tensor=t.ap().tensor, offset=offset, ap=ap)

    with tile.TileContext(nc) as tc:
        with (
            tc.tile_pool(name="persist", bufs=1) as pp,
            tc.tile_pool(name="ctxp", bufs=2) as ctxp,
            tc.tile_pool(name="xp", bufs=3) as xp,
            tc.tile_pool(name="stg", bufs=3) as stg,
            tc.tile_pool(name="small", bufs=2) as sp,
            tc.tile_pool(name="ps", bufs=2, space="PSUM") as ps,
        ):
            # ---- constants / params ----
            w4 = pp.tile([128, NK], F32, tag="w4")
            nc.sync.dma_start(out=w4[:], in_=w4_in[:])
            ab = pp.tile([NK, 1], F32, tag="ab")
            nc.sync.dma_start(out=ab[:], in_=ab_in[:])
            kern32 = pp.tile([C, NK * 9 * C], F32, tag="kern")
            nc.sync.dma_start(out=kern32[:], in_=kern_in[:])
            gam = pp.tile([C, 1], F32, tag="gam")
            nc.sync.dma_start(out=gam[:], in_=gam_in[:])
            bet = pp.tile([C, 1], F32, tag="bet")
            nc.sync.dma_start(out=bet[:], in_=bet_in[:])
            ones4 = pp.tile([NK, 1], F32, tag="ones4")
            nc.vector.memset(ones4[:], 1.0)
            eps32 = pp.tile([C, 1], F32, tag="eps")
            nc.vector.memset(eps32[:], BN_EPS)

            wrep = [pp.tile([128, 9 * C], BF16, tag=f"wrep{s}", name=f"wrep{s}") for s in range(SPC)]
            strip = pp.tile([128, SPC * n_iters * NBAND, 6], F32, tag="strip")

            # ---- per-sample attention -> dynamic conv weights ----
            for s in range(SPC):
                pstrip = sp.tile([128, n_ctx_tiles], F32, tag="pstrip")
                for t in range(n_ctx_tiles):
                    cxt = ctxp.tile([128, ctx_tile_f], BF16, tag="cxt")
                    nc.sync.dma_start(
                        out=cxt[:],
                        in_=dram_ap(
                            ctx_in,
                            s * chw + t * ctx_tile_f,
                            [[qhw, 4], [hw, C], [1, ctx_tile_f]],
                        ),
                    )
                    nc.vector.reduce_sum(
                        out=pstrip[:, t : t + 1], in_=cxt[:], axis=mybir.AxisListType.X
                    )
                pooled = sp.tile([128, 1], F32, tag="pooled")
                nc.vector.reduce_sum(out=pooled[:], in_=pstrip[:], axis=mybir.AxisListType.X)

                # logits[n] = sum_{q,c} w4[(q,c),n] * pooled[(q,c)]   (w4 pre-scaled 1/HW)
                pl = ps.tile([NK, 1], F32, tag="bank0")
                nc.tensor.matmul(pl[:], w4[:], pooled[:], start=True, stop=True)
                exps = sp.tile([NK, 1], F32, tag="exps")
                nc.scalar.activation(out=exps[:], in_=pl[:], func=AF.Exp, bias=ab[:], scale=1.0)
                ssum = ps.tile([1, 1], F32, tag="bank1")
                nc.tensor.matmul(ssum[:], ones4[:], exps[:], start=True, stop=True)
                recip = sp.tile([1, 1], F32, tag="recip")
                nc.vector.reciprocal(out=recip[:], in_=ssum[:])

                nc.sync.dma_start(out=exps_d[:], in_=exps[:])
                nc.sync.dma_start(out=recip_d[:], in_=recip[:])
                attn32 = sp.tile([C, NK], F32, tag="attn32")
                nc.sync.dma_start(
                    out=attn32[:], in_=dram_ap(exps_d, 0, [[0, C], [1, NK]])
                )
                recip32 = sp.tile([C, 1], F32, tag="recip32")
                nc.sync.dma_start(
                    out=recip32[:], in_=dram_ap(recip_d, 0, [[0, C], [1, 1]])
                )

                # dyn32[ci,(t,co)] = (sum_n attn_n * kern32[ci, n*(9C)+(t,co)]) * recip
                dyn32 = sp.tile([C, 9 * C], F32, tag="dyn32")
                nc.vector.tensor_scalar(
                    dyn32[:], kern32[:, 0 : 9 * C], attn32[:, 0:1], None, op0=ALU.mult
                )
                for n in range(1, NK):
                    nc.vector.scalar_tensor_tensor(
                        out=dyn32[:],
                        in0=kern32[:, n * 9 * C : (n + 1) * 9 * C],
                        scalar=attn32[:, n : n + 1],
                        in1=dyn32[:],
                        op0=ALU.mult,
                        op1=ALU.add,
                    )
                nc.vector.tensor_scalar(dyn32[:], dyn32[:], recip32[:], None, op0=ALU.mult)
                dynbf = sp.tile([C, 9 * C], BF16, tag="dynbf")
                nc.vector.tensor_copy(out=dynbf[:], in_=dyn32[:])
                nc.sync.dma_start(out=dyn_d[:], in_=dynbf[:])
                nc.sync.dma_start(
                    out=wrep[s][:],
                    in_=dram_ap(
                        dyn_d, 0, [[0, NBAND], [9 * C, C], [1, 9 * C]]
                    ),
                )

            # ---- conv pass over one sample; phase 1 collects stats, phase 2
            #      applies BN affine and writes out ----
            def conv_pass(s, phase2, scale128=None, shift128=None):
                xbase = PAD + s * chw
                for it in range(n_iters):
                    r0 = it * RPI
                    xw = xp.tile([128, NR, BW], BF16, tag="xw")
                    slot_lo = 1 if it == 0 else 0
                    slot_hi = NR - 1 if it == n_iters - 1 else NR
                    nrows = slot_hi - slot_lo
                    for b in range(NBAND):
                        nc.sync.dma_start(
                            out=xw[32 * b : 32 * b + 32, slot_lo:slot_hi, :],
                            in_=dram_ap(
                                x_in,
                                xbase + 128 * b - 1 + (r0 - 1 + slot_lo) * W,
                                [[hw, C], [W, nrows], [1, BW]],
                            ),
                        )
                    # zero the out-of-image columns/rows the halo DMA mis-read
                    nc.gpsimd.memset(xw[0:C, :, 0:1], 0.0)
                    nc.gpsimd.memset(xw[96:128, :, BW - 1 : BW], 0.0)
                    if it == 0:
                        nc.vector.memset(xw[:, 0:1, :], 0.0)
                    if it == n_iters - 1:
                        nc.vector.memset(xw[:, NR - 1 : NR, :], 0.0)

                    banks = [ps.tile([128, 512], F32, tag=f"bank{i}", name=f"bank{i}") for i in range(NBAND)]
                    for t9 in range(9):
                        kh, kw = divmod(t9, 3)
                        for i in range(NBAND):
                            for j in range(NBAND):
                                nc.tensor.matmul(
                                    banks[i][32 * j : 32 * j + 32, :],
                                    wrep[s][32 * i : 32 * i + 32, 32 * t9 : 32 * t9 + 32],
                                    xw[32 * i : 32 * i + 32, 4 * j + kh : 4 * j + kh + 4, kw : kw + 128],
                                    start=(t9 == 0),
                                    stop=(t9 == 8),
                                    tile_position=(32 * i, 32 * j),
                                    # the sim's group check is partition-blind
                                    # and trips on disjoint slices of one bank;
                                    # per-element has_written handles this
                                    skip_group_check=True,
                                )
                    if not phase2:
                        for i in range(NBAND):
                            st = (s * n_iters + it) * NBAND + i
                            nc.vector.bn_stats(out=strip[:, st, :], in_=banks[i][:])
                    else:
                        # stage free dims = (ri:4, band:4, w:128) -> row-major
                        # contiguous 2048-element rows on the DRAM side
                        stage = stg.tile([128, NBAND, NBAND, 128], F32, tag="stage")
                        for i in range(NBAND):
                            nc.scalar.activation(
                                out=stage[:, :, i, :],
                                in_=banks[i][:].rearrange("p (r w) -> p r w", w=128),
                                func=AF.Identity,
                                bias=shift128[:],
                                scale=scale128[:],
                            )
                        nc.sync.dma_start(
                            out=dram_ap(
                                out_ext,
                                s * chw + r0 * W,
                                [[4 * W, NBAND], [hw, C], [1, 4 * W]],
                            ),
                            in_=stage[:],
                        )

            for s in range(SPC):
                conv_pass(s, phase2=False)

            # ---- global BN statistics ----
            mv = sp.tile([128, 2], F32, tag="mv")
            nc.vector.bn_aggr(out=mv[:], in_=strip[:])
            msq = sp.tile([128, 1], F32, tag="msq")
            nc.vector.tensor_mul(msq[:], mv[:, 0:1], mv[:, 0:1])
            ar_sb = sp.tile([128, 2], F32, tag="ar_sb")
            nc.vector.tensor_scalar(
                ar_sb[:, 0:1], mv[:, 0:1], float(n_loc), None, op0=ALU.mult
            )
            nc.vector.scalar_tensor_tensor(
                out=ar_sb[:, 1:2],
                in0=mv[:, 1:2],
                scalar=1.0,
                in1=msq[:],
                op0=ALU.mult,
                op1=ALU.add,
            )
            nc.vector.tensor_scalar(
                ar_sb[:, 1:2], ar_sb[:, 1:2], float(n_loc), None, op0=ALU.mult
            )
            nc.sync.dma_start(out=ar_in_d[:], in_=ar_sb[:])
            nc.gpsimd.collective_compute(
                "AllReduce",
                ALU.add,
                replica_groups=[list(range(N_CORES))],
                ins=[ar_in_d[:]],
                outs=[ar_out_d[:]],
            )
            gsum = sp.tile([C, 2, 4], F32, tag="gsum")
            nc.sync.dma_start(
                out=gsum[:], in_=dram_ap(ar_out_d, 0, [[2, C], [1, 2], [64, 4]])
            )
            gs = sp.tile([C, 2], F32, tag="gs")
            nc.vector.reduce_sum(out=gs[:], in_=gsum[:], axis=mybir.AxisListType.X)
            mean_g = sp.tile([C, 1], F32, tag="mean_g")
            nc.vector.tensor_scalar(mean_g[:], gs[:, 0:1], 1.0 / n_glob, None, op0=ALU.mult)
            var_g = sp.tile([C, 1], F32, tag="var_g")
            nc.vector.tensor_scalar(var_g[:], gs[:, 1:2], 1.0 / n_glob, None, op0=ALU.mult)
            msg = sp.tile([C, 1], F32, tag="msg")
            nc.vector.tensor_mul(msg[:], mean_g[:], mean_g[:])
            nc.vector.tensor_sub(var_g[:], var_g[:], msg[:])
            std = sp.tile([C, 1], F32, tag="std")
            nc.scalar.activation(out=std[:], in_=var_g[:], func=AF.Sqrt, bias=eps32[:], scale=1.0)
            inv = sp.tile([C, 1], F32, tag="inv")
            nc.vector.reciprocal(out=inv[:], in_=std[:])
            ssb32 = sp.tile([C, 2], F32, tag="ssb32")
            nc.vector.tensor_mul(ssb32[:, 0:1], inv[:], gam[:])
            nc.vector.tensor_mul(ssb32[:, 1:2], mean_g[:], ssb32[:, 0:1])
            nc.vector.tensor_sub(ssb32[:, 1:2], bet[:], ssb32[:, 1:2])
            nc.sync.dma_start(out=ss_d[:], in_=ssb32[:])
            ssb = pp.tile([128, 2], F32, tag="ssb")
            nc.sync.dma_start(
                out=ssb[:],
                in_=dram_ap(ss_d, 0, [[0, 4], [2, C], [1, 2]]),
            )

            for s in range(SPC):
                conv_pass(s, phase2=True, scale128=ssb[:, 0:1], shift128=ssb[:, 1:2])

    nc.compile()
    return nc


_CACHE = {}


def get_nc(h=H):
    if h not in _CACHE:
        _CACHE[h] = _build(h)
    return _CACHE[h]


def make_in_maps(x, context_features, kernels, attn_w, attn_b, bn_gamma, bn_beta, h=H):
    import ml_dtypes

    bf = ml_dtypes.bfloat16
    x = np.ascontiguousarray(x, dtype=np.float32).astype(bf)
    ctx = np.ascontiguousarray(context_features, dtype=np.float32).astype(bf)
    kern = np.ascontiguousarray(
        np.transpose(np.asarray(kernels, np.float32), (2, 0, 3, 4, 1)).reshape(C, NK * 9 * C)
    )
    w4 = np.ascontiguousarray(
        np.tile(np.asarray(attn_w, np.float32).T / float(h * W), (4, 1))
    )
    ab = np.asarray(attn_b, np.float32).reshape(NK, 1)
    gam = np.asarray(bn_gamma, np.float32).reshape(C, 1)
    bet = np.asarray(bn_beta, np.float32).reshape(C, 1)
    pad = np.zeros(PAD, bf)
    in_maps = []
    for c in range(N_CORES):
        xs = x[SPC * c : SPC * (c + 1)].ravel()
        in_maps.append(
            {
                "x": np.concatenate([pad, xs, pad]),
                "ctx": ctx[SPC * c : SPC * (c + 1)].ravel(),
                "kern": kern,
                "w4": w4,
                "ab": ab,
                "gam": gam,
                "bet": bet,
            }
        )
    return in_maps


def kernel(x, context_features, kernels, attn_w, attn_b, bn_gamma, bn_beta):
    nc = get_nc(H)
    in_maps = make_in_maps(
        x, context_features, kernels, attn_w, attn_b, bn_gamma, bn_beta, H
    )
    res = run_bass_kernel_spmd(nc, in_maps, list(range(N_CORES)))
    out = np.concatenate(
        [res.results[c]["out"].reshape(SPC, C, H, W) for c in range(N_CORES)], axis=0
    )
    return out


# revision 21
# speedup vs baseline: 2.8423x; 2.8423x over previous
"""Trainium2 Bass kernel for nn_DynamicConvolution (dense_cnn).

Reference computation (per batch of 16 samples):
  pooled = mean(context, HW) -> logits = pooled@attn_w.T + b -> softmax over 4
  dyn_k[b] = sum_n attn[b,n] * kernels[n]          (per-sample 3x3 conv weights)
  out = conv2d(x, dyn_k, SAME)                     (32->32ch, 512x512)
  out = batchnorm(out, batch stats over (B,H,W))

Sharding: data-parallel, 2 samples per NeuronCore across 8 cores. BN batch
statistics are all-reduced on-device (tiny [128,2] collective).

Conv strategy: tap-shift matmuls. x is staged in SBUF as
[128 partitions = 4 W-bands x 32 ch, 18 rows x 130 cols (1-col halo)].
Each of the 9 taps is one K=32 matmul per (W-band i, row-quad j) accumulated
in PSUM via tile_position=(32i,32j) -- 16 sub-array-concurrent matmuls per
tap round, bf16 operands (fp32r forbids col-group tiling; bf16 also
halves the x/ctx HBM traffic -- the host passes x/ctx pre-cast to bf16).

BN is two-pass without storing an intermediate: phase 1 computes conv and
bn_stats only (no output write), stats are all-reduced, phase 2 recomputes
conv and applies the affine BN epilogue on the Scalar engine while copying
PSUM->SBUF, then DMAs out. Total HBM traffic/core ~= ctx 64MB + 2x x-reads
(72MB each, halo) + out 64MB.
"""

import numpy as np

import concourse.bacc as bacc
import concourse.bass as bass
import concourse.mybir as mybir
import concourse.tile as tile
from concourse.bass_utils import run_bass_kernel_spmd

F32 = mybir.dt.float32
BF16 = mybir.dt.bfloat16
AF = mybir.ActivationFunctionType
ALU = mybir.AluOpType

B, C, H, W, NK = 16, 32, 512, 512, 4
BN_EPS = 1e-5
N_CORES = 8
SPC = B // N_CORES          # samples per core
PAD = 516                   # element pad around flat x (OOB-safe halo reads)
RPI = 16                    # output rows per super-iteration
NBAND = 4                   # W-bands (128 cols each)
BW = 130                    # band width incl 1-col halo each side
NR = RPI + 2                # input rows per window


def _build(h=H, parts="all"):
    """Build + compile the 8-core Bacc program. h parametrizes image height
    (h=H for real runs, smaller for simulation tests)."""
    n_iters = h // RPI
    chw = C * h * W
    hw = h * W
    qhw = hw // 4                  # ctx row-quarter size per channel
    n_ctx_tiles = max(1, qhw // 8192)
    ctx_tile_f = qhw // n_ctx_tiles
    n_loc = SPC * n_iters * NBAND * 512   # elements per (quad,ch) stat partition
    n_glob = float(B * h * W)

    nc = bacc.Bacc(
        "TRN2",
        target_bir_lowering=False,
        debug=False,
        enable_asserts=False,
        num_devices=N_CORES,
    )

    x_in = nc.declare_dram_parameter("x", [SPC * 128 * (h + 2) * BW], BF16, isOutput=False)
    ctx_in = nc.declare_dram_parameter("ctx", [SPC * chw], BF16, isOutput=False)
    kern_in = nc.declare_dram_parameter("kern", [C, NK * 9 * C], F32, isOutput=False)
    w4_in = nc.declare_dram_parameter("w4", [128, NK], F32, isOutput=False)
    ab_in = nc.declare_dram_parameter("ab", [NK, 1], F32, isOutput=False)
    gam_in = nc.declare_dram_parameter("gam", [C, 1], F32, isOutput=False)
    bet_in = nc.declare_dram_parameter("bet", [C, 1], F32, isOutput=False)
    out_ext = nc.declare_dram_parameter("out", [SPC * n_iters * 128 * 2048], F32, isOutput=True)

    # internal DRAM
    exps_d = nc.dram_tensor("exps_d", [NK], F32)
    recip_d = nc.dram_tensor("recip_d", [1], F32)
    dyn_d = nc.dram_tensor("dyn_d", [C * 9 * C], BF16)
    ss_d = nc.dram_tensor("ss_d", [C, 2], F32)
    ar_in_d = nc.dram_tensor("ar_in", [128, 2], F32)
    ar_out_d = nc.dram_tensor("ar_out", [128, 2], F32, addr_space="Shared")

    def dram_ap(t, offset, ap):
        return bass.AP(tensor=t.ap().tensor, offset=offset, ap=ap)

    with tile.TileContext(nc) as tc:
        with (
            tc.tile_pool(name="persist", bufs=1) as pp,
            tc.tile_pool(name="ctxp", bufs=2) as ctxp,
            tc.tile_pool(name="xp", bufs=3) as xp,
            tc.tile_pool(name="stg", bufs=3) as stg,
            tc.tile_pool(name="small", bufs=2) as sp,
            tc.tile_pool(name="ps", bufs=2, space="PSUM") as ps,
        ):
            # ---- constants / params ----
            w4 = pp.tile([128, NK], F32, tag="w4")
            nc.sync.dma_start(out=w4[:], in_=w4_in[:])
            ab = pp.tile([NK, 1], F32, tag="ab")
            nc.sync.dma_start(out=ab[:], in_=ab_in[:])
            kern32 = pp.tile([C, NK * 9 * C], F32, tag="kern")
            nc.sync.dma_start(out=kern32[:], in_=kern_in[:])
            gam = pp.tile([C, 1], F32, tag="gam")
            nc.sync.dma_start(out=gam[:], in_=gam_in[:])
            bet = pp.tile([C, 1], F32, tag="bet")
            nc.sync.dma_start(out=bet[:], in_=bet_in[:])
            ones4 = pp.tile([NK, 1], F32, tag="ones4")
            nc.vector.memset(ones4[:], 1.0)
            eps32 = pp.tile([C, 1], F32, tag="eps")
            nc.vector.memset(eps32[:], BN_EPS)

            wrep = [pp.tile([128, 9 * C], BF16, tag=f"wrep{s}", name=f"wrep{s}") for s in range(SPC)]
            strip = pp.tile([128, SPC * n_iters * NBAND, 6], F32, tag="strip")

            # ---- per-sample attention -> dynamic conv weights ----
            for s in range(SPC):
                pstrip = sp.tile([128, n_ctx_tiles], F32, tag="pstrip")
                for t in range(n_ctx_tiles):
                    cxt = ctxp.tile([128, ctx_tile_f], BF16, tag="cxt")
                    nc.sync.dma_start(
                        out=cxt[:],
                        in_=dram_ap(
                            ctx_in,
                            s * chw + t * ctx_tile_f,
                            [[qhw, 128], [1, ctx_tile_f]],
                        ),
                    )
                    nc.vector.reduce_sum(
                        out=pstrip[:, t : t + 1],
                        in_=cxt[:, 0:64] if parts == "ctxdma" else cxt[:],
                        axis=mybir.AxisListType.X,
                    )
                pooled = sp.tile([128, 1], F32, tag="pooled")
                nc.vector.reduce_sum(out=pooled[:], in_=pstrip[:], axis=mybir.AxisListType.X)
                if parts in ("ctxonly", "ctxdma"):
                    nc.vector.bn_stats(out=strip[:, s : s + 1, :], in_=pooled[:].broadcast_to((128, 2)))
                    continue

                # logits[n] = sum_{q,c} w4[(q,c),n] * pooled[(q,c)]   (w4 pre-scaled 1/HW)
                pl = ps.tile([NK, 1], F32, tag="bank0")
                nc.tensor.matmul(pl[:], w4[:], pooled[:], start=True, stop=True)
                exps = sp.tile([NK, 1], F32, tag="exps")
                nc.scalar.activation(out=exps[:], in_=pl[:], func=AF.Exp, bias=ab[:], scale=1.0)
                ssum = ps.tile([1, 1], F32, tag="bank1")
                nc.tensor.matmul(ssum[:], ones4[:], exps[:], start=True, stop=True)
                recip = sp.tile([1, 1], F32, tag="recip")
                nc.vector.reciprocal(out=recip[:], in_=ssum[:])

                nc.sync.dma_start(out=exps_d[:], in_=exps[:])
                nc.sync.dma_start(out=recip_d[:], in_=recip[:])
                attn32 = sp.tile([C, NK], F32, tag="attn32")
                nc.sync.dma_start(
                    out=attn32[:], in_=dram_ap(exps_d, 0, [[0, C], [1, NK]])
                )
                recip32 = sp.tile([C, 1], F32, tag="recip32")
                nc.sync.dma_start(
                    out=recip32[:], in_=dram_ap(recip_d, 0, [[0, C], [1, 1]])
                )

                # dyn32[ci,(t,co)] = (sum_n attn_n * kern32[ci, n*(9C)+(t,co)]) * recip
                dyn32 = sp.tile([C, 9 * C], F32, tag="dyn32")
                nc.vector.tensor_scalar(
                    dyn32[:], kern32[:, 0 : 9 * C], attn32[:, 0:1], None, op0=ALU.mult
                )
                for n in range(1, NK):
                    nc.vector.scalar_tensor_tensor(
                        out=dyn32[:],
                        in0=kern32[:, n * 9 * C : (n + 1) * 9 * C],
                        scalar=attn32[:, n : n + 1],
                        in1=dyn32[:],
                        op0=ALU.mult,
                        op1=ALU.add,
                    )
                nc.vector.tensor_scalar(dyn32[:], dyn32[:], recip32[:], None, op0=ALU.mult)
                dynbf = sp.tile([C, 9 * C], BF16, tag="dynbf")
                nc.vector.tensor_copy(out=dynbf[:], in_=dyn32[:])
                nc.sync.dma_start(out=dyn_d[:], in_=dynbf[:])
                nc.sync.dma_start(
                    out=wrep[s][:],
                    in_=dram_ap(
                        dyn_d, 0, [[0, NBAND], [9 * C, C], [1, 9 * C]]
                    ),
                )

            # ---- conv pass over one sample; phase 1 collects stats, phase 2
            #      applies BN affine and writes out ----
            def conv_pass(s, phase2, scale128=None, shift128=None):
                xbase = s * 128 * (h + 2) * BW
                for it in range(n_iters):
                    r0 = it * RPI
                    xw = xp.tile([128, NR, BW], BF16, tag="xw")
                    nc.sync.dma_start(
                        out=xw[:],
                        in_=dram_ap(
                            x_in,
                            xbase + r0 * BW,
                            [[(h + 2) * BW, 128], [1, NR * BW]],
                        ),
                    )

                    banks = [ps.tile([128, 512], F32, tag=f"bank{i}", name=f"bank{i}") for i in range(NBAND)]
                    for t9 in range(9):
                        kh, kw = divmod(t9, 3)
                        for i in range(NBAND):
                            for j in range(NBAND):
                                nc.tensor.matmul(
                                    banks[i][32 * j : 32 * j + 32, :],
                                    wrep[s][32 * i : 32 * i + 32, 32 * t9 : 32 * t9 + 32],
                                    xw[32 * i : 32 * i + 32, 4 * j + kh : 4 * j + kh + 4, kw : kw + 128],
                                    start=(t9 == 0),
                                    stop=(t9 == 8),
                                    tile_position=(32 * i, 32 * j),
                                    # the sim's group check is partition-blind
                                    # and trips on disjoint slices of one bank;
                                    # per-element has_written handles this
                                    skip_group_check=True,
                                )
                    if not phase2:
                        for i in range(NBAND):
                            st = (s * n_iters + it) * NBAND + i
                            nc.vector.bn_stats(out=strip[:, st, :], in_=banks[i][:])
                    else:
                        # stage free dims = (ri:4, band:4, w:128) -> row-major
                        # contiguous 2048-element rows on the DRAM side
                        stage = stg.tile([128, NBAND, NBAND, 128], F32, tag="stage")
                        for i in range(NBAND):
                            nc.scalar.activation(
                                out=stage[:, :, i, :],
                                in_=banks[i][:].rearrange("p (r w) -> p r w", w=128),
                                func=AF.Identity,
                                bias=shift128[:],
                                scale=scale128[:],
                            )
                        nc.sync.dma_start(
                            out=dram_ap(
                                out_ext,
                                (s * n_iters + it) * 128 * 2048,
                                [[2048, 128], [1, 2048]],
                            ),
                            in_=stage[:],
                        )

            if parts in ("all", "ph1", "noar"):
                for s in range(SPC):
                    conv_pass(s, phase2=False)
            if parts == "attn":
                for s in range(SPC):
                    nc.vector.bn_stats(out=strip[:, s : s + 1, :], in_=wrep[s][:, 0:256].bitcast(F32))

            # ---- global BN statistics ----
            mv = sp.tile([128, 2], F32, tag="mv")
            nc.vector.bn_aggr(out=mv[:], in_=strip[:])
            msq = sp.tile([128, 1], F32, tag="msq")
            nc.vector.tensor_mul(msq[:], mv[:, 0:1], mv[:, 0:1])
            ar_sb = sp.tile([128, 2], F32, tag="ar_sb")
            nc.vector.tensor_scalar(
                ar_sb[:, 0:1], mv[:, 0:1], float(n_loc), None, op0=ALU.mult
            )
            nc.vector.scalar_tensor_tensor(
                out=ar_sb[:, 1:2],
                in0=mv[:, 1:2],
                scalar=1.0,
                in1=msq[:],
                op0=ALU.mult,
                op1=ALU.add,
            )
            nc.vector.tensor_scalar(
                ar_sb[:, 1:2], ar_sb[:, 1:2], float(n_loc), None, op0=ALU.mult
            )
            nc.sync.dma_start(out=ar_in_d[:], in_=ar_sb[:])
            if parts != "noar":
                nc.gpsimd.collective_compute(
                    "AllReduce",
                    ALU.add,
                    replica_groups=[list(range(N_CORES))],
                    ins=[ar_in_d[:]],
                    outs=[ar_out_d[:]],
                )
            ar_src = ar_in_d if parts == "noar" else ar_out_d
            gsum = sp.tile([C, 2, 4], F32, tag="gsum")
            nc.sync.dma_start(
                out=gsum[:], in_=dram_ap(ar_src, 0, [[2, C], [1, 2], [64, 4]])
            )
            gs = sp.tile([C, 2], F32, tag="gs")
            nc.vector.reduce_sum(out=gs[:], in_=gsum[:], axis=mybir.AxisListType.X)
            mean_g = sp.tile([C, 1], F32, tag="mean_g")
            nc.vector.tensor_scalar(mean_g[:], gs[:, 0:1], 1.0 / n_glob, None, op0=ALU.mult)
            var_g = sp.tile([C, 1], F32, tag="var_g")
            nc.vector.tensor_scalar(var_g[:], gs[:, 1:2], 1.0 / n_glob, None, op0=ALU.mult)
            msg = sp.tile([C, 1], F32, tag="msg")
            nc.vector.tensor_mul(msg[:], mean_g[:], mean_g[:])
            nc.vector.tensor_sub(var_g[:], var_g[:], msg[:])
            std = sp.tile([C, 1], F32, tag="std")
            nc.scalar.activation(out=std[:], in_=var_g[:], func=AF.Sqrt, bias=eps32[:], scale=1.0)
            inv = sp.tile([C, 1], F32, tag="inv")
            nc.vector.reciprocal(out=inv[:], in_=std[:])
            ssb32 = sp.tile([C, 2], F32, tag="ssb32")
            nc.vector.tensor_mul(ssb32[:, 0:1], inv[:], gam[:])
            nc.vector.tensor_mul(ssb32[:, 1:2], mean_g[:], ssb32[:, 0:1])
            nc.vector.tensor_sub(ssb32[:, 1:2], bet[:], ssb32[:, 1:2])
            nc.sync.dma_start(out=ss_d[:], in_=ssb32[:])
            ssb = pp.tile([128, 2], F32, tag="ssb")
            nc.sync.dma_start(
                out=ssb[:],
                in_=dram_ap(ss_d, 0, [[0, 4], [2, C], [1, 2]]),
            )

            if parts in ("all",):
                for s in range(SPC):
                    conv_pass(s, phase2=True, scale128=ssb[:, 0:1], shift128=ssb[:, 1:2])

    nc.compile()
    return nc


_CACHE = {}


def get_nc(h=H, parts="all"):
    if (h, parts) not in _CACHE:
        _CACHE[(h, parts)] = _build(h, parts)
    return _CACHE[(h, parts)]


def prep_x(x, h):
    """[B,C,h,W] -> per-sample band windows [B, 128=(band,ch), h+2, 130] bf16
    with zero-padded row/col halos baked in."""
    import ml_dtypes

    bf = ml_dtypes.bfloat16
    b = x.shape[0]
    xp_ = np.zeros((b, NBAND, C, h + 2, BW), bf)
    xpad = np.pad(np.asarray(x, np.float32), ((0, 0), (0, 0), (0, 0), (1, 1))).astype(bf)
    for bb in range(NBAND):
        xp_[:, bb, :, 1 : h + 1, :] = xpad[:, :, :, 128 * bb : 128 * bb + BW]
    return xp_.reshape(b, 128 * (h + 2) * BW)


def prep_ctx(ctx, h):
    import ml_dtypes

    bf = ml_dtypes.bfloat16
    b = ctx.shape[0]
    qh = h // 4
    return (
        np.asarray(ctx, np.float32)
        .astype(bf)
        .reshape(b, C, 4, qh * W)
        .transpose(0, 2, 1, 3)
        .reshape(b, C * h * W)
    )


def unpermute_out(dev, h):
    """[n_iters*128*2048] device blocks -> [C, h, W] per sample."""
    n_iters = h // RPI
    d = dev.reshape(n_iters, NBAND, C, NBAND, NBAND, 128)  # it, j, c, ri, i, w
    return np.ascontiguousarray(d.transpose(2, 0, 1, 3, 4, 5)).reshape(C, h, W)


def make_in_maps(x, context_features, kernels, attn_w, attn_b, bn_gamma, bn_beta, h=H):
    x = prep_x(np.ascontiguousarray(x, dtype=np.float32), h)
    ctx = prep_ctx(np.ascontiguousarray(context_features, dtype=np.float32), h)
    kern = np.ascontiguousarray(
        np.transpose(np.asarray(kernels, np.float32), (2, 0, 3, 4, 1)).reshape(C, NK * 9 * C)
    )
    w4 = np.ascontiguousarray(
        np.tile(np.asarray(attn_w, np.float32).T / float(h * W), (4, 1))
    )
    ab = np.asarray(attn_b, np.float32).reshape(NK, 1)
    gam = np.asarray(bn_gamma, np.float32).reshape(C, 1)
    bet = np.asarray(bn_beta, np.float32).reshape(C, 1)
    in_maps = []
    for c in range(N_CORES):
        in_maps.append(
            {
                "x": x[SPC * c : SPC * (c + 1)].ravel(),
                "ctx": ctx[SPC * c : SPC * (c + 1)].ravel(),
                "kern": kern,
                "w4": w4,
                "ab": ab,
                "gam": gam,
                "bet": bet,
            }
        )
    return in_maps


def kernel(x, context_features, kernels, attn_w, attn_b, bn_gamma, bn_beta):
    nc = get_nc(H)
    in_maps = make_in_maps(
        x, context_features, kernels, attn_w, attn_b, bn_gamma, bn_beta, H
    )
    res = run_bass_kernel_spmd(nc, in_maps, list(range(N_CORES)))
    per_sample = SPC * (H // RPI) * 0 + (H // RPI) * 128 * 2048
    out = np.stack(
        [
            unpermute_out(res.results[c]["out"].reshape(SPC, per_sample)[s_], H)
            for c in range(N_CORES)
            for s_ in range(SPC)
        ],
        axis=0,
    )
    return out
